# revision 8
# baseline (speedup 1.0000x reference)
"""Trainium2 Bass kernel for nn_DataEmbedder (embedding_lookup).

Forward pass of a tabular data embedder:
  - dataset [64, 4096, 12] f32: cols 0-3 are raw categorical ids (stored as
    floats), cols 4-11 are numeric features.
  - For each categorical col k: ids -> lut_k remap -> emb_k gather.
  - Output [64, 4096, 128] = concat(emb0[32], emb1[64], emb2[16], emb3[8],
    numeric[8]).

Strategy (data-parallel over batch: 8 cores x 8 batch rows):
  Per-token gathers use the GPSIMD `dma_gather` custom instruction (the only
  primitive that batches thousands of indexed-row reads in one instruction;
  `indirect_dma_start` consumes just one offset per partition). dma_gather
  requires 256-byte gather elements and int16 indices wrapped in 16
  partitions (replicated across the 8 Q7 cores), so:

  Stage A (per core, once):
    - pemb_k [8192, 64] f32: plain strided-DMA copy of emb_k into 256B-padded
      rows (pad cols/rows garbage, never read).
    - lut_k loaded in wrapped-int16 layout, then cemb_k = pemb_k[lut_k] via
      one dma_gather + writeback: the lut-composed table with 256B rows.
  Stage B (4 chunks of 8192 tokens):
    - id cols loaded in wrapped layout [16, 2048, 4] -> DVE-convert to int16
      per table -> replicate to 128 partitions (3 partition-doubling DMAs).
    - one dma_gather per (chunk, table) from cemb_k -> staging tile
      [128, 64, 64]; token n of the chunk lands at [n%128, n//128].
    - DVE copies assemble the valid d columns + numeric cols into the
      [128, 64, 128] output tile; one store DMA per chunk writes rows
      n = i*128+p.
"""

import numpy as np

B, T = 64, 4096
NCORES = 8
BC = B // NCORES            # batch rows per core
N = BC * T                  # 32768 tokens per core
NCOLS = 12
VOCABS = [1000, 5000, 200, 50]
DIMS = [32, 64, 16, 8]
OFF = [0, 32, 96, 112]      # output column offset of each embedding block
NUM_OFF = 120               # numeric features start col
DOUT = 128
NCHUNK = 4
CH = N // NCHUNK            # 8192 tokens per chunk
IPP = CH // 128             # 64 out slots per partition per chunk
SPC = CH // 16              # 512 wrapped idx slots per chunk
PAD = 64                    # padded row length (f32) = 256B
PROWS = 8192                # padded table rows (>= CH for AP-span checks)
VPAD = [((v + 127) // 128) * 128 for v in VOCABS]   # 1024, 5120, 256, 128
LUT_SLOT_OFF = [0, 64, 384, 400]                    # int16 slots in lut region
LUT_REGION = 512                                    # total lut slots (408 pad 512)
TOK_SLOTS = 4 * (N // 16)                           # 4 tables x 2048
W16 = TOK_SLOTS + LUT_REGION                        # big16 tile free dim

_CACHE = {}


def _build_program():
    from contextlib import ExitStack

    import concourse.bacc as bacc
    import concourse.tile as tile
    from concourse import mybir
    from concourse.tile import add_dep_helper

    F32, I32, I16 = mybir.dt.float32, mybir.dt.int32, mybir.dt.int16

    nc = bacc.Bacc("TRN2", target_bir_lowering=False, debug=False,
                   num_devices=NCORES)
    ds = nc.dram_tensor("ds", [N, NCOLS], F32, kind="ExternalInput")
    out = nc.dram_tensor("out", [N, DOUT], F32, kind="ExternalOutput")
    embs = [
        nc.dram_tensor(f"emb{k}", [VOCABS[k], DIMS[k]], F32, kind="ExternalInput")
        for k in range(4)
    ]
    luts = [
        nc.dram_tensor(f"lut{k}", [VOCABS[k]], I32, kind="ExternalInput")
        for k in range(4)
    ]
    pembs = [nc.dram_tensor(f"pemb{k}", [PROWS, PAD], F32) for k in range(4)]
    cembs = [nc.dram_tensor(f"cemb{k}", [PROWS, PAD], F32) for k in range(4)]

    with tile.TileContext(nc) as tc:
        with ExitStack() as ctx:
            sm_pool = ctx.enter_context(tc.tile_pool(name="small", bufs=1))
            comp_pool = ctx.enter_context(tc.tile_pool(name="comp", bufs=1))
            nds_pool = ctx.enter_context(tc.tile_pool(name="nds", bufs=1))
            g_pool = ctx.enter_context(tc.tile_pool(name="gt", bufs=3))
            o_pool = ctx.enter_context(tc.tile_pool(name="ot", bufs=2))

            # ---------- Stage A: padded copies + lut composition ----------
            pemb_cp = []
            for k in range(4):
                w = nc.sync.dma_start(
                    out=pembs[k].ap()[: VOCABS[k], : DIMS[k]],
                    in_=embs[k].ap(),
                )
                pemb_cp.append(w)

            # wrapped int16 index tile: [128, 4*2048 tokens | 512 lut slots]
            big16 = sm_pool.tile([128, W16], I16, name="big16")
            # lut region: zero the pad slots (stray big positive idx would
            # read out of bounds; 0 is always safe)
            nc.vector.memset(big16[:16, TOK_SLOTS:], 0)

            # load luts int32 in wrapped layout, convert to int16
            wlut32s = []
            for k in range(4):
                nslot = VPAD[k] // 16
                wlut32 = sm_pool.tile([16, nslot], I32, name=f"wlut32_{k}")
                nc.vector.memset(wlut32[:], 0)
                V = VOCABS[k]
                m, tail = V // 16, V % 16
                nc.sync.dma_start(
                    out=wlut32[:, :m],
                    in_=luts[k].ap()[: 16 * m].rearrange("(s r) -> r s", r=16),
                )
                if tail:
                    nc.sync.dma_start(
                        out=wlut32[:tail, m : m + 1],
                        in_=luts[k].ap()[16 * m :].rearrange("(s r) -> r s", r=tail),
                    )
                lo = TOK_SLOTS + LUT_SLOT_OFF[k]
                nc.vector.tensor_copy(
                    out=big16[:16, lo : lo + nslot], in_=wlut32[:]
                )
                wlut32s.append(wlut32)

            # ---------- Stage B prep: token id cols + numeric ----------
            widx = sm_pool.tile([16, N // 16, 4], F32, name="widx")
            nc.sync.dma_start(
                out=widx[:],
                in_=ds.ap()[:, 0:4].rearrange("(s r) k -> r s k", r=16),
            )
            for k in range(4):
                nc.vector.tensor_copy(
                    out=big16[:16, k * (N // 16) : (k + 1) * (N // 16)],
                    in_=widx[:, :, k],
                )

            # replicate partitions 0:16 -> 0:128 by doubling
            nc.sync.dma_start(out=big16[16:32, :], in_=big16[0:16, :])
            nc.sync.dma_start(out=big16[32:64, :], in_=big16[0:32, :])
            nc.sync.dma_start(out=big16[64:128, :], in_=big16[0:64, :])

            # lut composition: cemb_k = pemb_k[lut_k]
            wb = []
            for k in range(4):
                nslot = VPAD[k] // 16
                lo = TOK_SLOTS + LUT_SLOT_OFF[k]
                comp_t = comp_pool.tile(
                    [128, VPAD[k] // 128, PAD], F32, name=f"comp_t{k}"
                )
                cg = nc.gpsimd.dma_gather(
                    comp_t[:],
                    pembs[k].ap(),
                    big16[:, lo : lo + nslot],
                    VPAD[k],
                    VPAD[k],
                    PAD,
                    single_packet=False,
                )
                add_dep_helper(cg.ins, pemb_cp[k].ins, reason=f"pemb{k} RAW")
                w = nc.sync.dma_start(
                    out=cembs[k].ap()[: VPAD[k], :].rearrange(
                        "(i p) d -> p i d", p=128
                    ),
                    in_=comp_t[:],
                )
                wb.append(w)

            # numeric features, already in the [p=n%128, i=n//128] layout
            nds = nds_pool.tile([128, N // 128, 8], F32, name="nds")
            nc.sync.dma_start(
                out=nds[:],
                in_=ds.ap()[:, 4:NCOLS].rearrange("(i p) k -> p i k", p=128),
            )

            # ---------- Stage B: per-chunk gather + assemble + store ----------
            for c in range(NCHUNK):
                o_t = o_pool.tile([128, IPP, DOUT], F32, name="o_t")
                nc.vector.tensor_copy(
                    out=o_t[:, :, NUM_OFF:],
                    in_=nds[:, c * IPP : (c + 1) * IPP, :],
                )
                for k in range(4):
                    g_t = g_pool.tile([128, IPP, PAD], F32, name="g_t")
                    so = k * (N // 16) + c * SPC
                    gi = nc.gpsimd.dma_gather(
                        g_t[:],
                        cembs[k].ap(),
                        big16[:, so : so + SPC],
                        CH,
                        CH,
                        PAD,
                        single_packet=False,
                    )
                    add_dep_helper(gi.ins, wb[k].ins, reason=f"cemb{k} RAW")
                    nc.vector.tensor_copy(
                        out=o_t[:, :, OFF[k] : OFF[k] + DIMS[k]],
                        in_=g_t[:, :, : DIMS[k]],
                    )
                nc.sync.dma_start(
                    out=out.ap()[c * CH : (c + 1) * CH, :].rearrange(
                        "(i p) f -> p i f", p=128
                    ),
                    in_=o_t[:],
                )
    nc.compile()
    return nc


def get_program():
    if "nc" not in _CACHE:
        _CACHE["nc"] = _build_program()
    return _CACHE["nc"]


def make_in_maps(inputs):
    dataset = np.asarray(inputs["dataset"], dtype=np.float32)
    in_maps = []
    for i in range(NCORES):
        m = {
            "ds": np.ascontiguousarray(
                dataset[i * BC : (i + 1) * BC].reshape(N, NCOLS)
            )
        }
        for k in range(4):
            m[f"emb{k}"] = np.ascontiguousarray(inputs[f"emb{k}"], dtype=np.float32)
            m[f"lut{k}"] = np.ascontiguousarray(inputs[f"lut{k}"], dtype=np.int32)
        in_maps.append(m)
    return in_maps


def kernel(**inputs):
    from concourse.bass_utils import run_bass_kernel_spmd

    nc = get_program()
    in_maps = make_in_maps(inputs)
    res = run_bass_kernel_spmd(nc, in_maps, list(range(NCORES))).results
    outs = [np.asarray(res[i]["out"]).reshape(BC, T, DOUT) for i in range(NCORES)]
    return np.concatenate(outs, axis=0)
